# revision 20
# baseline (speedup 1.0000x reference)
"""VQ-codebook linear layer on 8 Trainium2 NeuronCores.

Computes  out = x @ W^T + bias  where  W = centroids[labels].reshape(4096, 4096).

Strategy (tensor-parallel over out_features, per the sharding hint:
"shard labels/centroid-gathered weight along out_features, replicate x"):
  - each core owns a 512-row slice of W (out_features / 8)
  - x is replicated to every core (transposed to [in, tok] fp16 on host)
  - the centroid-gathered W^T shard is pre-gathered on host (a previous
    session measured on-device ap_gather dequant ~1 ms slower: ~60
    cycles/index on GPSIMD, cannot hide under the matmul stream)
  - matmul in fp16 (same PE rate as bf16, ~8x less rounding error for
    unit-scale data; fp32 PSUM accumulation): lhsT = W^T tile
    [128 in, 128 out] (stationary), rhs = x^T tile [128 in, 512 tok]
    (moving), PSUM accumulates over the 32 k-tiles; bias added on the
    PSUM->SBUF copy (DVE tensor_scalar_add)
  - per-core output is [512 out, 8192 tok]; host concatenates and
    transposes back to [4, 2048, 4096]

PE roofline per core: 4 m-tiles x 32 k-tiles x 8192 tokens = 1.05M moving
rows at 1 row/cycle @ 2.4 GHz = 437 us. DMA per core ~76 MiB at ~360 GB/s
= ~220 us, hides under the matmul stream. Cost-model sim of this program:
444.6 us single-shot, PE busy 98.2% -- within 1.7% of the fp16 PE roofline.
fp8 (DoubleRow, 2x PE rate) was evaluated and rejected: e4m3 quantization
of both operands gives rel err ~0.21 on this data (tolerance 2e-2) because
the K=4096 accumulation amplifies per-product quantization noise.
"""

import numpy as np
import ml_dtypes
import concourse.bass as bass
import concourse.tile as tile
from concourse import bacc, mybir, library_config
from concourse import bass_utils

TOK = 8192          # 4 * 2048 tokens
DIN = 4096
DOUT = 4096
BD = 16             # block dim (centroid vector length)
NCLUST = 256        # codebook size
N_CORES = 8
KT = DIN // 128              # 32 k-tiles
NTOK = 512                   # tokens per matmul (moving free dim)

# core grid: osplit-way over out_features x tsplit-way over tokens.
# (8, 1) = pure tensor-parallel (x replicated, 76 MiB DMA/core);
# (4, 2) = 2D (x half-replicated, 48 MiB DMA/core -- less HBM power,
# less thermal throttle in sustained runs; same PE work either way)
# VQ_OSPLIT env override exists only for A/B experiments (exp2d.py).
import os as _os
OSPLIT = int(_os.environ.get("VQ_OSPLIT", "8"))
TSPLIT = N_CORES // OSPLIT
OSH = DOUT // OSPLIT         # out features per core
MT = OSH // 128              # m-tiles per core
TOKC = TOK // TSPLIT         # tokens per core
NT = TOKC // NTOK            # n-tiles per core

DEVICE_GATHER = False  # host-gathered W^T shard (see module docstring)
PRECISION = "f16"      # "f16" | "bf16" — low-precision matmul dtype
OUT_F16 = True         # write the [512, 8192] output shard as fp16 (host
                       # upconverts): 8 MiB/core less device DMA; costs
                       # ~1.3e-3 extra rel err

# scheduling knobs (A/B-tested via exp.py)
PSUM_BUFS = 4          # PSUM accumulator ring (max 8 banks)
X_BUFS = 2             # x token-tile prefetch depth (32 KB/partition each)
WT_BUFS = 1            # W^T shard buffers (8 KB/partition per m-tile);
                       # 2 overlaps next-iteration weight DMA in For_i runs
OUT_BUFS = 3           # output staging tiles (1 KB/partition each)
NGROUP = 1             # token-tiles interleaved per weight pass
OUT_DMA = "sync"       # "sync" | "scalar" — HWDGE ring for output stores
DRAIN = "vector"       # "vector" | "split" — PSUM drain engine(s)
STARTUP_SPLIT = 8      # >1: split the first x token-tile DMA into this many
                       # k-chunks on the sync ring and move the W^T loads to
                       # the scalar HWDGE ring (wt0 k-chunked), so the first
                       # matmul starts after ~0.5 MiB of DMA instead of
                       # ~8 MiB. Cost-model sim: 469.4 -> 444.6 us
                       # single-shot (PE busy 93.3% -> 98.2%); measured
                       # -17 us/iter in the For_i A/B (round 3)


def build_nc(device_gather: bool = DEVICE_GATHER, repeat: int = 1,
             precision: str = PRECISION, out_f16: bool = OUT_F16,
             psum_bufs: int = PSUM_BUFS, x_bufs: int = X_BUFS,
             wt_bufs: int = WT_BUFS, out_bufs: int = OUT_BUFS,
             ngroup: int = NGROUP, out_dma: str = OUT_DMA,
             drain: str = DRAIN, startup_split: int = STARTUP_SPLIT):
    """Build and bacc-compile the per-core bass program (SPMD: all cores run
    the same program on different DRAM inputs)."""
    import contextlib

    nc = bacc.Bacc("TRN2", target_bir_lowering=False, debug=False,
                   enable_asserts=True, num_devices=N_CORES)
    f32, i16 = mybir.dt.float32, mybir.dt.int16
    bf16 = mybir.dt.float16 if precision == "f16" else mybir.dt.bfloat16

    xT_ap = nc.dram_tensor("xT", [DIN, TOKC], bf16, kind="ExternalInput").ap()
    bias_ap = nc.dram_tensor("biasc", [128, MT], f32, kind="ExternalInput").ap()
    if device_gather:
        table_ap = nc.dram_tensor("table", [128, NCLUST], f32,
                                  kind="ExternalInput").ap()
        idx_ap = nc.dram_tensor("idx", [128, MT * KT * 128 // 16], i16,
                                kind="ExternalInput").ap()
    else:
        wt_ap = nc.dram_tensor("wt", [128, MT * KT * 128], bf16,
                               kind="ExternalInput").ap()
    odt = bf16 if out_f16 else f32
    out_ap = nc.dram_tensor("out", [OSH, TOKC], odt, kind="ExternalOutput").ap()

    # DRAM views
    # xT [DIN, TOKC] -> [128 p, KT, TOKC]
    xview = xT_ap.rearrange("(kt p) t -> p kt t", p=128)
    # out [OSH, TOKC] -> [MT, 128 p, TOKC]
    oview = out_ap.rearrange("(mt p) t -> mt p t", p=128)

    out_eng = nc.scalar if out_dma == "scalar" else nc.sync

    with tile.TileContext(nc) as tc:
        with contextlib.ExitStack() as ctx:
            const_pool = ctx.enter_context(tc.tile_pool(name="const", bufs=1))
            wt_pool = ctx.enter_context(tc.tile_pool(name="wt", bufs=wt_bufs))
            x_pool = ctx.enter_context(tc.tile_pool(name="x", bufs=x_bufs))
            psum_pool = ctx.enter_context(
                tc.tile_pool(name="psum", bufs=psum_bufs, space="PSUM"))
            out_pool = ctx.enter_context(tc.tile_pool(name="ob", bufs=out_bufs))
            if device_gather:
                g_pool = ctx.enter_context(tc.tile_pool(name="g", bufs=2))
                nc.gpsimd.load_library(library_config.ap_gather)

            bias_t = const_pool.tile([128, MT], f32)
            nc.sync.dma_start(bias_t[:], bias_ap[:])

            if device_gather:
                table_t = const_pool.tile([128, NCLUST], f32)
                nc.sync.dma_start(table_t[:], table_ap[:])
                idx_t = const_pool.tile([128, MT * KT * 128 // 16], i16)
                nc.sync.dma_start(idx_t[:], idx_ap[:])

            # startup_split>1 also moves the weight loads to the scalar
            # (ACT) HWDGE ring so they don't queue behind the x loads
            wt_eng = nc.scalar if startup_split > 1 else nc.sync

            def load_wt(wt_t, m, chunks=1):
                if chunks == 1:
                    wt_eng.dma_start(wt_t[m][:], wt_ap[:, bass.ts(m, KT * 128)])
                else:
                    ck = KT * 128 // chunks
                    for c in range(chunks):
                        wt_eng.dma_start(
                            wt_t[m][:, bass.ts(c, ck)],
                            wt_ap[:, bass.ds(m * KT * 128 + c * ck, ck)])

            def dequant():
                # W^T per-core shard, fp16, free layout (m, kt, o'): 8 KB/part
                # per m-tile. In startup_split mode wt0 is k-chunked so the
                # first LDWEIGHTS can start after ~0.25 MiB of DMA.
                wt_t = [wt_pool.tile([128, KT * 128], bf16, tag=f"wt{m}",
                                     name=f"wt{m}")
                        for m in range(MT)]
                if device_gather:
                    nidx_chunk = KT * 128
                    for m in range(MT):
                        g = g_pool.tile([128, nidx_chunk], f32, tag="g")
                        nc.gpsimd.ap_gather(
                            g[:], table_t[:],
                            idx_t[:, bass.ts(m, nidx_chunk // 16)],
                            channels=128, num_elems=NCLUST, d=1,
                            num_idxs=nidx_chunk,
                        )
                        nc.vector.tensor_copy(wt_t[m][:], g[:])
                else:
                    for m in range(MT):
                        load_wt(wt_t, m,
                                chunks=4 if (startup_split > 1 and m == 0)
                                else 1)
                return wt_t

            def drain_one(idx, ps, m, n):
                ob = out_pool.tile([128, NTOK], odt, tag="ob", name="ob")
                if drain == "split" and idx % 2 == 1:
                    # scalar (ACT) engine drain: out = Identity(ps + bias)
                    nc.scalar.activation(
                        ob[:], ps[:], mybir.ActivationFunctionType.Identity,
                        bias=bias_t[:, m:m + 1])
                else:
                    nc.vector.tensor_scalar_add(ob[:], ps[:],
                                                bias_t[:, m:m + 1])
                out_eng.dma_start(oview[m, :, bass.ts(n, NTOK)], ob[:])

            def body(wt_t):
                di = 0
                for g in range(NT // ngroup):
                    xts = []
                    for i in range(ngroup):
                        n = g * ngroup + i
                        xt = x_pool.tile([128, KT, NTOK], bf16, tag="xn",
                                         name="xt")
                        if n == 0 and startup_split > 1 and not device_gather:
                            ck = KT // startup_split
                            for c in range(startup_split):
                                nc.sync.dma_start(
                                    xt[:, bass.ts(c, ck), :],
                                    xview[:, bass.ts(c, ck),
                                          bass.ts(n, NTOK)])
                        else:
                            nc.sync.dma_start(
                                xt[:], xview[:, :, bass.ts(n, NTOK)])
                        xts.append(xt)
                    for m in range(MT):
                        pss = [psum_pool.tile([128, NTOK], f32, tag="ps",
                                              name="ps")
                               for _ in range(ngroup)]
                        for kt in range(KT):
                            for i in range(ngroup):
                                nc.tensor.matmul(
                                    pss[i][:],
                                    lhsT=wt_t[m][:, bass.ts(kt, 128)],
                                    rhs=xts[i][:, kt, :],
                                    start=(kt == 0), stop=(kt == KT - 1))
                        for i, ps in enumerate(pss):
                            drain_one(di, ps, m, g * ngroup + i)
                            di += 1

            if repeat == 1:
                body(dequant())
            else:
                with tc.For_i(0, repeat, 1):
                    body(dequant())

    nc.compile()
    return nc


def _host_prep(x, centroids, labels, bias, device_gather: bool = DEVICE_GATHER,
               precision: str = PRECISION):
    """Relayout inputs for the per-core DRAM tensors."""
    lpdt = np.float16 if precision == "f16" else ml_dtypes.bfloat16
    labels2d = np.asarray(labels).reshape(DOUT, DIN // BD)   # [out, block]
    cent = np.asarray(centroids, dtype=np.float32)           # [256, 16]
    # cast before transposing: elementwise astype commutes with .T and the
    # fp16 transpose-copy moves half the bytes of the fp32 one
    xT = np.ascontiguousarray(
        np.asarray(x).reshape(TOK, DIN).astype(lpdt).T)      # [DIN, TOK]

    xT_shards = [xT] if TSPLIT == 1 else [
        np.ascontiguousarray(xT[:, t * TOKC:(t + 1) * TOKC])
        for t in range(TSPLIT)]

    in_maps = []
    for c in range(N_CORES):
        to, mo = c // OSPLIT, c % OSPLIT
        osl = slice(mo * OSH, (mo + 1) * OSH)
        bias_c = np.ascontiguousarray(
            np.asarray(bias, dtype=np.float32)[osl].reshape(MT, 128).T)
        m = {"xT": xT_shards[to], "biasc": bias_c}
        if device_gather:
            # per-partition centroid-column table: row 16b+j = centroids[:, j]
            table = np.ascontiguousarray(np.tile(cent.T, (8, 1)))  # [128, 256]
            l4 = labels2d[osl].reshape(MT, 128, KT, 8)   # [m, o', kt, b]
            seq = l4.transpose(3, 0, 2, 1).reshape(8, MT * KT * 128)  # [b, i]
            wrapped = seq.reshape(8, MT * KT * 128 // 16, 16)
            idx = np.ascontiguousarray(
                wrapped.transpose(0, 2, 1).reshape(128, MT * KT * 128 // 16)
            ).astype(np.int16)
            m["table"] = table
            m["idx"] = idx
        else:
            # host dequant of the W^T shard in (m, kt, o') free layout:
            # wt[16b+j, m*KT*128 + kt*128 + o'] = cent[labels2d[osl][m*128+o', 8kt+b], j]
            w = cent[labels2d[osl]]                  # [512, 256, 16]
            w = w.reshape(MT, 128, KT, 8, BD)        # [m, o', kt, b, j]
            wt = w.transpose(3, 4, 0, 2, 1).reshape(128, MT * KT * 128)
            m["wt"] = np.ascontiguousarray(wt).astype(lpdt)
        in_maps.append(m)
    return in_maps


_CACHE = {}


def kernel(x, centroids, labels, bias):
    key = (DEVICE_GATHER,)
    if key not in _CACHE:
        _CACHE[key] = build_nc(DEVICE_GATHER, repeat=1)
    nc = _CACHE[key]
    in_maps = _host_prep(x, centroids, labels, bias, DEVICE_GATHER)
    res = bass_utils.run_bass_kernel_spmd(
        nc, in_maps, core_ids=list(range(N_CORES)))
    # assemble [TOK, DOUT] directly: one transposing-cast copy per shard
    # instead of concatenate + full-matrix transpose materialization
    out = np.empty((TOK, DOUT), np.float32)
    for c in range(N_CORES):
        to, mo = c // OSPLIT, c % OSPLIT
        out[to * TOKC:(to + 1) * TOKC,
            mo * OSH:(mo + 1) * OSH] = res.results[c]["out"].T
    return out.reshape(4, 2048, DOUT)
